# revision 8
# baseline (speedup 1.0000x reference)
"""Trainium2 Bass kernel for ColumnMixedPrecisionLinear (v7).

Computes out[b,s,o] = bias[o] + sum_i x_i[b,s,:] @ (wq_i * s_i[:,None]).T
where x is [4, 2048, 4096] fp32, wq_i are [4096, 1024] int8 slices of the
weight along the input dim, s_i are per-output-channel scales.

Strategy: data-parallel over tokens across 8 NeuronCores. Each core gets
1024 tokens of x (flattened [8192, 4096]) and the full weights, and computes
its [1024, 4096] output shard. No cross-device reduction needed.

Design — keep the PE streaming back-to-back, nothing else on the critical
path (PE floor: 2048 matmuls x 512 cols @ 2.4 GHz ~ 445 us):
  - x pre-transposed/packed/cast to bf16 on the HOST:
    xb[p, j, b, tl] = bf16(x[t=j*128+tl, d=b*128+p]); one 1MB DMA per
    j-tile. No device-side transposes.
  - W pre-transposed/packed on the host, int8:
    wp[p, c*16384 + b*512 + ol] = Wfull[d=b*128+p, o=c*512+ol]; per o-chunk
    one contiguous 16KB/partition SWDGE DMA with int8->bf16 cast (exact).
  - Scales packed PER CHUNK (sc[p, c, i, ol] = s_i[c*512+ol]) so chunk 0's
    dequant only waits on a 512KB load, not the whole scale tensor (v3
    lost 35us of lead-in to that).
  - Dequant: DVE broadcast multiply per (chunk, slice).
  - Main loop per (ochunk c, ttile j): 32 accumulating matmuls
    [128d,128t] @ [128d,512o] into one PSUM bank; ACT drains psum -> bf16;
    per-(c,j) 128KB output stores keep the tail short.
  - bias added on the HOST (output-linear); out stored bf16, upcast on host.
"""

import numpy as np
import ml_dtypes

import concourse.bass as bass
import concourse.mybir as mybir
import concourse.tile as tile
from concourse import bacc
from concourse.bass_utils import run_bass_kernel_spmd

P = 128
N_CORES = 8
B, S = 4, 2048
D_IN_SLICE = 1024
N_SLICES = 4
D = D_IN_SLICE * N_SLICES      # 4096 contraction dim
O = 4096                       # out features
T = (B * S) // N_CORES         # 1024 tokens per core

T_TILES = T // P               # 8
D_BLKS = D // P                # 32
D_BLKS_SLICE = D_IN_SLICE // P # 8
O_CHUNK = 512
O_CHUNKS = O // O_CHUNK        # 8

BF16 = mybir.dt.bfloat16
FP32 = mybir.dt.float32
INT8 = mybir.dt.int8

VER = 7  # bumped per kernel-body revision to dodge stale NEFF cache hits


def build_nc():
    nc = bacc.Bacc(None, target_bir_lowering=False)
    vtag = nc.dram_tensor("vtag", [1, VER + 1], FP32, kind="ExternalOutput")

    # packed bf16 x: [p, j*4096 + b*128 + tl]
    xb_in = nc.dram_tensor("xb", [P, T_TILES * D_BLKS * P], BF16,
                           kind="ExternalInput")
    # packed int8 W: [p, c*16384 + b*512 + ol]
    wp_in = nc.dram_tensor("wp", [P, O_CHUNKS * D_BLKS * O_CHUNK], INT8,
                           kind="ExternalInput")
    # per-chunk partition-broadcast scales: [p, c, i, ol] = s_i[c*512+ol]
    sc_in = nc.dram_tensor("sc", [P, O_CHUNKS, N_SLICES, O_CHUNK], BF16,
                           kind="ExternalInput")
    out = nc.dram_tensor("out", [T, O], BF16, kind="ExternalOutput")

    with tile.TileContext(nc) as tc:
        with (
            tc.tile_pool(name="const", bufs=1) as const,
            tc.tile_pool(name="xres", bufs=1) as xres,
            # bufs is PER TAG: 8 persistent tags x 1 buf = 32KB/partition
            tc.tile_pool(name="sc", bufs=1) as sc_pool,
            tc.tile_pool(name="wt", bufs=2) as wt_pool,
            tc.tile_pool(name="ostage", bufs=4) as ostage,
            tc.tile_pool(name="psm", bufs=7, space="PSUM") as psm,
            tc.tile_pool(name="psw", bufs=1, space="PSUM") as psw_pool,
        ):
            vt = const.tile([1, VER + 1], FP32)
            nc.vector.memset(vt[:], 0.0)
            nc.sync.dma_start(vtag[:], vt[:])

            # PE warmup: garbage matmuls during the load phase so the HAM
            # clock gate is at 2.4 GHz when the real matmuls start
            wa = const.tile([P, P], BF16)
            nc.vector.memset(wa[:], 0.5)
            wb = const.tile([P, O_CHUNK], BF16)
            nc.vector.memset(wb[:], 0.5)
            psw = psw_pool.tile([P, O_CHUNK], FP32, tag="psw")
            for _ in range(24):
                nc.tensor.matmul(psw[:], wa[:], wb[:], start=True, stop=True)

            xs = xres.tile([P, T_TILES, D_BLKS, P], BF16)
            scs = []
            for j in range(T_TILES):
                # x on the ACT HWDGE ring; sc on the sync ring
                sct = sc_pool.tile([P, N_SLICES, O_CHUNK], BF16, tag=f"sc{j}")
                nc.sync.dma_start(sct[:], sc_in[:, j, :, :])
                scs.append(sct)
                nc.scalar.dma_start(
                    xs[:, j, :, :],
                    xb_in[:, j * D_BLKS * P:(j + 1) * D_BLKS * P]
                    .rearrange("p (b tl) -> p b tl", b=D_BLKS),
                )

            for c in range(O_CHUNKS):
                wt = wt_pool.tile([P, D_BLKS, O_CHUNK], BF16, tag="wt")
                # int8 -> bf16 cast DMA (SWDGE). Chunk 0 is split per slice
                # so the first dequant only waits on 512KB; later chunks use
                # one DMA to keep fixed costs down and not starve x loads.
                if c == 0:
                    for i in range(N_SLICES):
                        base = i * D_BLKS_SLICE * O_CHUNK
                        nc.gpsimd.dma_start(
                            wt[:, i * D_BLKS_SLICE:(i + 1) * D_BLKS_SLICE, :],
                            wp_in[:, base:base + D_BLKS_SLICE * O_CHUNK]
                            .rearrange("p (b ol) -> p b ol", b=D_BLKS_SLICE),
                        )
                else:
                    nc.gpsimd.dma_start(
                        wt[:],
                        wp_in[:, c * D_BLKS * O_CHUNK:
                              (c + 1) * D_BLKS * O_CHUNK]
                        .rearrange("p (b ol) -> p b ol", b=D_BLKS),
                    )
                # dequant: per-slice broadcast multiply on DVE
                for i in range(N_SLICES):
                    nc.vector.tensor_tensor(
                        wt[:, i * D_BLKS_SLICE:(i + 1) * D_BLKS_SLICE, :],
                        wt[:, i * D_BLKS_SLICE:(i + 1) * D_BLKS_SLICE, :],
                        scs[c][:, i, None, :]
                        .to_broadcast((P, D_BLKS_SLICE, O_CHUNK)),
                        mybir.AluOpType.mult,
                    )

                for j in range(T_TILES):
                    ps = psm.tile([P, O_CHUNK], FP32, tag="ps")
                    for db in range(D_BLKS):
                        nc.tensor.matmul(
                            ps[:],
                            xs[:, j, db, :],
                            wt[:, db, :],
                            start=(db == 0),
                            stop=(db == D_BLKS - 1),
                        )
                    # drain on ACT: psum fp32 -> sbuf bf16
                    ob = ostage.tile([P, O_CHUNK], BF16, tag="ob")
                    nc.scalar.activation(
                        ob[:], ps[:], mybir.ActivationFunctionType.Copy
                    )
                    nc.sync.dma_start(
                        out[j * P:(j + 1) * P, c * O_CHUNK:(c + 1) * O_CHUNK],
                        ob[:],
                    )
    nc.compile()
    return nc


_NC_CACHE = None


def _get_nc():
    global _NC_CACHE
    if _NC_CACHE is None:
        _NC_CACHE = build_nc()
    return _NC_CACHE


def _prep_inputs(x, wqs, ss):
    xf = np.asarray(x, dtype=np.float32).reshape(B * S, D)

    # Wfull[d, o] = wq_{d // 1024}[o, d % 1024]
    wfull = np.empty((D, O), dtype=np.int8)
    for i in range(N_SLICES):
        wfull[i * D_IN_SLICE:(i + 1) * D_IN_SLICE, :] = np.asarray(wqs[i]).T
    # [d, o] -> [b, p, c, ol] -> [p, c, b, ol] -> [p, c*16384 + b*512 + ol]
    wp = np.ascontiguousarray(
        wfull.reshape(D_BLKS, P, O_CHUNKS, O_CHUNK).transpose(1, 2, 0, 3)
    ).reshape(P, O_CHUNKS * D_BLKS * O_CHUNK)

    # sc[p, c, i, ol] = s_i[c*512 + ol]
    sstack = np.stack([np.asarray(s, dtype=np.float32) for s in ss])  # [4, O]
    sc = np.ascontiguousarray(
        np.broadcast_to(
            sstack.reshape(N_SLICES, O_CHUNKS, O_CHUNK)
            .transpose(1, 0, 2)
            .astype(ml_dtypes.bfloat16)[None],
            (P, O_CHUNKS, N_SLICES, O_CHUNK),
        )
    )

    in_maps = []
    for c in range(N_CORES):
        xc = xf[c * T:(c + 1) * T]  # [1024, 4096]
        # [t, d] = [(j tl), (b p)] -> [p, j, b, tl]
        xbc = np.ascontiguousarray(
            xc.reshape(T_TILES, P, D_BLKS, P)
            .transpose(3, 0, 2, 1)
            .astype(ml_dtypes.bfloat16)
        ).reshape(P, T_TILES * D_BLKS * P)
        in_maps.append({"xb": xbc, "wp": wp, "sc": sc})
    return in_maps


def run_on_hw(x, wqs, ss, bias, **spmd_kwargs):
    """Run and return (out_full [B,S,O] fp32, BassKernelResults)."""
    nc = _get_nc()
    in_maps = _prep_inputs(x, wqs, ss)
    res = run_bass_kernel_spmd(nc, in_maps, core_ids=list(range(N_CORES)),
                               **spmd_kwargs)
    out = np.concatenate([r["out"] for r in res.results], axis=0)
    out = out.astype(np.float32) + np.asarray(bias, dtype=np.float32)
    return np.ascontiguousarray(out.reshape(B, S, O)), res


def kernel(x, wq0, s0, wq1, s1, wq2, s2, wq3, s3, bias):
    out, _ = run_on_hw(x, [wq0, wq1, wq2, wq3], [s0, s1, s2, s3], bias)
    return out


# revision 10
# speedup vs baseline: 1.0144x; 1.0144x over previous
"""Trainium2 Bass kernel for ColumnMixedPrecisionLinear (v8).

Computes out[b,s,o] = bias[o] + sum_i x_i[b,s,:] @ (wq_i * s_i[:,None]).T
where x is [4, 2048, 4096] fp32, wq_i are [4096, 1024] int8 slices of the
weight along the input dim, s_i are per-output-channel scales.

Strategy: data-parallel over tokens across 8 NeuronCores. Each core gets
1024 tokens of x (flattened [8192, 4096]) and the full weights, and computes
its [1024, 4096] output shard. No cross-device reduction needed.

Design — keep the PE streaming back-to-back, nothing else on the critical
path (PE floor: 2048 matmuls x 512 cols @ 2.4 GHz ~ 445 us):
  - x pre-transposed/packed/cast to bf16 on the HOST:
    xb[p, j, b, tl] = bf16(x[t=j*128+tl, d=b*128+p]); one 1MB DMA per
    j-tile. No device-side transposes.
  - W pre-transposed/packed on the host, int8:
    wp[p, c*16384 + b*512 + ol] = Wfull[d=b*128+p, o=c*512+ol]; per o-chunk
    one contiguous 16KB/partition SWDGE DMA with int8->bf16 cast (exact).
  - Scales packed PER CHUNK (sc[p, c, i, ol] = s_i[c*512+ol]) so chunk 0's
    dequant only waits on a 512KB load, not the whole scale tensor (v3
    lost 35us of lead-in to that).
  - Dequant: DVE broadcast multiply per (chunk, slice).
  - Main loop per (ochunk c, ttile j): 32 accumulating matmuls
    [128d,128t] @ [128d,512o] into one PSUM bank; ACT drains psum -> bf16;
    per-(c,j) 128KB output stores keep the tail short.
  - bias added on the HOST (output-linear); out stored bf16, upcast on host.
"""

import numpy as np
import ml_dtypes

import concourse.bass as bass
import concourse.mybir as mybir
import concourse.tile as tile
from concourse import bacc
from concourse.bass_utils import run_bass_kernel_spmd

P = 128
N_CORES = 8
B, S = 4, 2048
D_IN_SLICE = 1024
N_SLICES = 4
D = D_IN_SLICE * N_SLICES      # 4096 contraction dim
O = 4096                       # out features
T = (B * S) // N_CORES         # 1024 tokens per core

T_TILES = T // P               # 8
D_BLKS = D // P                # 32
D_BLKS_SLICE = D_IN_SLICE // P # 8
O_CHUNK = 512
O_CHUNKS = O // O_CHUNK        # 8

BF16 = mybir.dt.bfloat16
FP32 = mybir.dt.float32
INT8 = mybir.dt.int8

VER = 8  # bumped per kernel-body revision to dodge stale NEFF cache hits


def build_nc():
    nc = bacc.Bacc(None, target_bir_lowering=False)
    vtag = nc.dram_tensor("vtag", [1, VER + 1], FP32, kind="ExternalOutput")

    # packed bf16 x: [p, j*4096 + b*128 + tl]
    xb_in = nc.dram_tensor("xb", [P, T_TILES * D_BLKS * P], BF16,
                           kind="ExternalInput")
    # packed int8 W: [p, c*16384 + b*512 + ol]
    wp_in = nc.dram_tensor("wp", [P, O_CHUNKS * D_BLKS * O_CHUNK], INT8,
                           kind="ExternalInput")
    # per-chunk partition-broadcast scales: [p, c, i, ol] = s_i[c*512+ol]
    sc_in = nc.dram_tensor("sc", [P, O_CHUNKS, N_SLICES, O_CHUNK], BF16,
                           kind="ExternalInput")
    out = nc.dram_tensor("out", [T, O], BF16, kind="ExternalOutput")

    def load_w(ws, c, split):
        """Stage chunk c of W (raw int8) via the ACT HWDGE ring."""
        if split:
            for i in range(N_SLICES):
                base = c * D_BLKS * O_CHUNK + i * D_BLKS_SLICE * O_CHUNK
                nc.scalar.dma_start(
                    ws[:, i * D_BLKS_SLICE:(i + 1) * D_BLKS_SLICE, :],
                    wp_in[:, base:base + D_BLKS_SLICE * O_CHUNK]
                    .rearrange("p (b ol) -> p b ol", b=D_BLKS_SLICE),
                )
        else:
            nc.scalar.dma_start(
                ws[:],
                wp_in[:, c * D_BLKS * O_CHUNK:(c + 1) * D_BLKS * O_CHUNK]
                .rearrange("p (b ol) -> p b ol", b=D_BLKS),
            )

    with tile.TileContext(nc) as tc:
        with (
            tc.tile_pool(name="const", bufs=1) as const,
            tc.tile_pool(name="xres", bufs=1) as xres,
            # bufs is PER TAG: 8 persistent tags x 1 buf = 32KB/partition
            tc.tile_pool(name="sc", bufs=1) as sc_pool,
            tc.tile_pool(name="wstage", bufs=2) as ws_pool,
            tc.tile_pool(name="wt", bufs=2) as wt_pool,
            tc.tile_pool(name="ostage", bufs=4) as ostage,
            tc.tile_pool(name="psm", bufs=8, space="PSUM") as psm,
        ):
            vt = const.tile([1, VER + 1], FP32)
            nc.vector.memset(vt[:], 0.0)
            nc.sync.dma_start(vtag[:], vt[:])

            xs = xres.tile([P, T_TILES, D_BLKS, P], BF16)
            scs = [sc_pool.tile([P, N_SLICES, O_CHUNK], BF16, tag=f"sc{j}",
                                name=f"sc{j}") for j in range(T_TILES)]
            wss = [ws_pool.tile([P, D_BLKS, O_CHUNK], INT8, tag="ws",
                                name=f"ws{k}") for k in range(2)]

            def load_x(j):
                nc.scalar.dma_start(
                    xs[:, j, :, :],
                    xb_in[:, j * D_BLKS * P:(j + 1) * D_BLKS * P]
                    .rearrange("p (b tl) -> p b tl", b=D_BLKS),
                )

            # ACT-ring program order fixes the DMA FIFO: x0, W0(split), x1,
            # W1, x2..x7. sc loads ride the sync ring.
            for j in range(T_TILES):
                nc.sync.dma_start(scs[j][:], sc_in[:, j, :, :])
            load_x(0)
            load_w(wss[0], 0, split=True)
            load_x(1)
            load_w(wss[1], 1, split=False)
            for j in range(2, T_TILES):
                load_x(j)

            for c in range(O_CHUNKS):
                if c >= 2:
                    wss[c % 2] = ws_pool.tile([P, D_BLKS, O_CHUNK], INT8,
                                              tag="ws", name=f"ws{c}")
                    load_w(wss[c % 2], c, split=False)
                ws = wss[c % 2]
                # fused cast+dequant: int8 W x bf16 scale -> bf16, per slice
                wt = wt_pool.tile([P, D_BLKS, O_CHUNK], BF16, tag="wt")
                for i in range(N_SLICES):
                    nc.vector.tensor_tensor(
                        wt[:, i * D_BLKS_SLICE:(i + 1) * D_BLKS_SLICE, :],
                        ws[:, i * D_BLKS_SLICE:(i + 1) * D_BLKS_SLICE, :],
                        scs[c][:, i, None, :]
                        .to_broadcast((P, D_BLKS_SLICE, O_CHUNK)),
                        mybir.AluOpType.mult,
                    )

                for j in range(T_TILES):
                    ps = psm.tile([P, O_CHUNK], FP32, tag="ps")
                    for db in range(D_BLKS):
                        nc.tensor.matmul(
                            ps[:],
                            xs[:, j, db, :],
                            wt[:, db, :],
                            start=(db == 0),
                            stop=(db == D_BLKS - 1),
                        )
                    # drain on ACT: psum fp32 -> sbuf bf16
                    ob = ostage.tile([P, O_CHUNK], BF16, tag="ob")
                    nc.scalar.activation(
                        ob[:], ps[:], mybir.ActivationFunctionType.Copy
                    )
                    nc.sync.dma_start(
                        out[j * P:(j + 1) * P, c * O_CHUNK:(c + 1) * O_CHUNK],
                        ob[:],
                    )
    nc.compile()
    return nc


_NC_CACHE = None


def _get_nc():
    global _NC_CACHE
    if _NC_CACHE is None:
        _NC_CACHE = build_nc()
    return _NC_CACHE


def _prep_inputs(x, wqs, ss):
    xf = np.asarray(x, dtype=np.float32).reshape(B * S, D)

    # Wfull[d, o] = wq_{d // 1024}[o, d % 1024]
    wfull = np.empty((D, O), dtype=np.int8)
    for i in range(N_SLICES):
        wfull[i * D_IN_SLICE:(i + 1) * D_IN_SLICE, :] = np.asarray(wqs[i]).T
    # [d, o] -> [b, p, c, ol] -> [p, c, b, ol] -> [p, c*16384 + b*512 + ol]
    wp = np.ascontiguousarray(
        wfull.reshape(D_BLKS, P, O_CHUNKS, O_CHUNK).transpose(1, 2, 0, 3)
    ).reshape(P, O_CHUNKS * D_BLKS * O_CHUNK)

    # sc[p, c, i, ol] = s_i[c*512 + ol]
    sstack = np.stack([np.asarray(s, dtype=np.float32) for s in ss])  # [4, O]
    sc = np.ascontiguousarray(
        np.broadcast_to(
            sstack.reshape(N_SLICES, O_CHUNKS, O_CHUNK)
            .transpose(1, 0, 2)
            .astype(ml_dtypes.bfloat16)[None],
            (P, O_CHUNKS, N_SLICES, O_CHUNK),
        )
    )

    in_maps = []
    for c in range(N_CORES):
        xc = xf[c * T:(c + 1) * T]  # [1024, 4096]
        # [t, d] = [(j tl), (b p)] -> [p, j, b, tl]
        xbc = np.ascontiguousarray(
            xc.reshape(T_TILES, P, D_BLKS, P)
            .transpose(3, 0, 2, 1)
            .astype(ml_dtypes.bfloat16)
        ).reshape(P, T_TILES * D_BLKS * P)
        in_maps.append({"xb": xbc, "wp": wp, "sc": sc})
    return in_maps


def run_on_hw(x, wqs, ss, bias, **spmd_kwargs):
    """Run and return (out_full [B,S,O] fp32, BassKernelResults)."""
    nc = _get_nc()
    in_maps = _prep_inputs(x, wqs, ss)
    res = run_bass_kernel_spmd(nc, in_maps, core_ids=list(range(N_CORES)),
                               **spmd_kwargs)
    out = np.concatenate([r["out"] for r in res.results], axis=0)
    out = out.astype(np.float32) + np.asarray(bias, dtype=np.float32)
    return np.ascontiguousarray(out.reshape(B, S, O)), res


def kernel(x, wq0, s0, wq1, s1, wq2, s2, wq3, s3, bias):
    out, _ = run_on_hw(x, [wq0, wq1, wq2, wq3], [s0, s1, s2, s3], bias)
    return out
